# revision 34
# baseline (speedup 1.0000x reference)
"""Trainium2 Bass kernel for nn_CharEmbedding (ragged_sequence).

Computation (see reference):
    rep = concat([emb[first], emb[mid].sum(1), emb[last]], -1)   # [U, 3H]
    out = rep @ head_w + head_b                                  # [U, O]
    tok = out[inv_i].reshape(B, L, O); pad time by (1,1)         # [B, L+2, O]

Strategy: data-parallel over the ~19.9k USED unique tokens (inv_i hits
only 1-e^-1.09 of the 30k uniques; capacity padded to 20480 = 8 x 2560
per-core rows).  The device computes one 768-wide output row per used
unique token; the host expands rows to the 32768 positions with a
fancy-index (pure index math) and pads the time axis.

1. The 12 mid-char embeddings per token become a counts-matmul on PE:
   msum[feat, tok] = sum_v E[v, feat] * cnt[v, tok] with
   host-precomputed per-tile count matrices (fp8 moving operand -
   integer counts <= 12 are exact in e4m3; bf16 E as the stationary
   operand keeps full accuracy and Fast-Weight-Load enabled).
   (An fp8 DoubleRow base+residual scheme was A/B-benchmarked and is
   NOT faster on hardware - DoubleRow's LDWEIGHTS overhead eats the
   theoretical 2x - while costing accuracy margin.)

2. The head weight blocks for first/last are folded on the host into
   lookup tables  T0 = E @ W[:256] + b  and  T2 = E @ W[512:].
   first/last bypass the head matmul: one plain dma_gather per tile
   fetches their 768-wide output-space rows token-major, and the DVE
   adds them onto the head PSUM during evacuation.  Head matmul
   shrinks to the 2 mid-block K-chunks (bf16).

3. Output rows are stored in bf16 (halves store traffic; rel-err
   contribution ~0.4%) and widened to f32 on the host.

Per 512-token tile: one SWDGE dma_gather (1024 T0|T2 rows), one
counts DMA on the SP HWDGE FIFO, 64 DoubleRow matmuls + ACT psum
evacuation, 16 bf16 head matmuls, 12 DVE adds, 4 bf16 row stores on
the ACT HWDGE FIFO.  (A/B-benchmarked against split/quartered DMAs,
fused per-tile stores and gather pre-sums - the simple shape below
is the fastest on hardware.)
"""

import numpy as np
import ml_dtypes

import concourse.bacc as bacc
import concourse.mybir as mybir
import concourse.tile as tile
from concourse.bass_utils import run_bass_kernel_spmd

BF16 = ml_dtypes.bfloat16
F8 = ml_dtypes.float8_e4m3

# Problem constants (hardcoded per contract).
VOCAB = 4000
VOCAB_PAD = 4096
U = 30000
M = 12
H = 256
O = 768
B = 16
L = 2048
N_CORES = 8
# Only unique tokens actually referenced by inv_i are computed on device
# (~19.9k of 30k for 32768 uniform draws); the position expansion is a
# host-side row gather.  Capacity is padded to a fixed shape.
U_CAP = 20480                    # padded used-unique capacity (8 * 2560)
T_CORE = U_CAP // N_CORES        # 2560 unique tokens per core
TILE_T = 512                     # tokens per pipeline tile
KMID = 2                         # head K-chunks (mid block only)
VCH = VOCAB_PAD // 128           # 32 vocab chunks for the counts matmul
NPAIR = VCH // 2                 # DoubleRow chunk pairs
GROWS = 2 * TILE_T               # gathered rows per tile (first|last)
IDXC = GROWS // 16               # idx columns per tile
GR = GROWS // 128                # gather output ranks (8)

_NC_CACHE = {}


def _tiles(n_tiles):
    return [TILE_T] * n_tiles

# fp8e4m3 encodings of integers 0..15 (counts never exceed 12)
_F8_LUT = np.arange(16).astype(F8).view(np.uint8)


def build_nc(n_tiles=T_CORE // TILE_T, reps=1, skip=()):
    """Build (and compile) the per-core Bass module.

    Tokens handled = n_tiles * TILE_T.  All cores run the same program.
    reps > 1 wraps the pipeline in a For_i hardware loop (timing only).
    skip: ablation knobs for sim experiments.
    """
    import contextlib
    t_core = n_tiles * TILE_T
    tiles = _tiles(n_tiles)
    assert sum(tiles) == t_core

    nc = bacc.Bacc("TRN2", target_bir_lowering=False, debug=False)

    t02_d = nc.dram_tensor("t02", [2 * VOCAB_PAD, O], mybir.dt.bfloat16,
                           kind="ExternalInput")
    # bf16 E table, vocab-chunk-major
    ecnt_d = nc.dram_tensor("ecnt", [128, VCH * H], mybir.dt.bfloat16,
                            kind="ExternalInput")
    wts_d = nc.dram_tensor("wts", [128, KMID * O], mybir.dt.bfloat16,
                           kind="ExternalInput")
    idx_d = nc.dram_tensor("idx", [128, t_core // 8], mybir.dt.int16,
                           kind="ExternalInput")
    cnt_d = nc.dram_tensor("cnt", [128, VCH * t_core],
                           mybir.dt.float8e4, kind="ExternalInput")
    out_d = nc.dram_tensor("out", [t_core, O], mybir.dt.bfloat16,
                           kind="ExternalOutput")

    with tile.TileContext(nc) as tc:
        with (
            tc.tile_pool(name="const", bufs=1) as cpool,
            tc.tile_pool(name="gath", bufs=2) as gpool,
            tc.tile_pool(name="cnts", bufs=2) as cntpool,
            tc.tile_pool(name="mids", bufs=2) as mpool,
            tc.tile_pool(name="outs", bufs=4) as opool,
            tc.tile_pool(name="psum_m", bufs=2, space="PSUM") as pmpool,
            tc.tile_pool(name="psum_h", bufs=3, space="PSUM") as phpool,
        ):
            # ---- resident constants ----
            idx_t = cpool.tile([128, t_core // 8], mybir.dt.int16)
            nc.sync.dma_start(out=idx_t[:], in_=idx_d[:])
            ecnt = cpool.tile([128, VCH, H], mybir.dt.bfloat16)
            nc.sync.dma_start(out=ecnt[:], in_=ecnt_d[:].rearrange(
                "p (c f) -> p c f", c=VCH))
            wts = cpool.tile([128, KMID, O], mybir.dt.bfloat16)
            nc.sync.dma_start(out=wts[:], in_=wts_d[:].rearrange(
                "p (c o) -> p c o", c=KMID))

            rep_ctx = tc.For_i(0, reps, 1) if reps > 1 else contextlib.nullcontext()
            with rep_ctx:
             tok0 = 0
             for t, tt in enumerate(tiles):
                # ---- counts tile DMA (quarters: DR pairs consume chunk
                # pairs in order, so the matmul starts after 1/4) ----
                cnt = cntpool.tile([128, VCH, tt], mybir.dt.float8e4,
                                   tag="c")
                s0 = VCH * tok0
                nc.sync.dma_start(
                    out=cnt[:],
                    in_=cnt_d[:, s0:s0 + VCH * tt].rearrange(
                        "p (c n) -> p c n", c=VCH))

                # ---- plain gather first|last rows of T0|T2, token-major --
                grows = 2 * tt
                gr = grows // 128
                gfl = gpool.tile([128, gr, O], mybir.dt.bfloat16, tag="g")
                col0 = tok0 // 8
                if "gather" not in skip:
                    nc.gpsimd.dma_gather(
                        gfl[:], t02_d[:], idx_t[:, col0:col0 + grows // 16],
                        grows, grows, O,
                        transpose=False,
                        single_packet=False, queue_num=0)
                else:
                    nc.vector.memset(gfl[:], 0.0)

                # ---- mid-sum as bf16 x fp8 counts matmul (FWL weights) ----
                msum = mpool.tile([128, 2, tt], mybir.dt.bfloat16)
                for f2 in range(2):
                    psm = pmpool.tile([128, tt], mybir.dt.float32)
                    fsl = slice(f2 * 128, (f2 + 1) * 128)
                    for c in range(VCH):
                        nc.tensor.matmul(
                            psm[:], ecnt[:, c, fsl], cnt[:, c, :],
                            start=(c == 0), stop=(c == VCH - 1))
                    nc.scalar.copy(msum[:, f2, :], psm[:])

                # ---- mid head matmuls + gather-row evacuate + store ----
                for m in range(tt // 128):
                    tok = slice(m * 128, (m + 1) * 128)
                    ps_a = phpool.tile([128, 512], mybir.dt.float32)
                    ps_b = phpool.tile([128, 256], mybir.dt.float32)
                    for ps, osl in ((ps_a, slice(0, 512)), (ps_b, slice(512, O))):
                        for c in range(KMID):
                            nc.tensor.matmul(
                                ps[:], msum[:, c, tok], wts[:, c, osl],
                                start=(c == 0), stop=(c == KMID - 1))
                    o_sb = opool.tile([128, O], mybir.dt.bfloat16)
                    # o = head_mid + T0[first] (+bias, folded) + T2[last]
                    nc.vector.tensor_add(o_sb[:, 0:512], ps_a[:],
                                         gfl[:, m, 0:512])
                    nc.vector.tensor_add(o_sb[:, 512:O], ps_b[:],
                                         gfl[:, m, 512:O])
                    nc.vector.tensor_add(o_sb[:], o_sb[:], gfl[:, gr // 2 + m, :])
                    row = tok0 + m * 128
                    nc.scalar.dma_start(out=out_d[row:row + 128, :],
                                        in_=o_sb[:])
                tok0 += tt

    nc.compile()
    return nc


def _get_nc(n_tiles=T_CORE // TILE_T):
    key = n_tiles
    if key not in _NC_CACHE:
        _NC_CACHE[key] = build_nc(n_tiles=n_tiles)
    return _NC_CACHE[key]


def _wrap_idx(stream):
    """Pack an index stream into the SWDGE gather layout: idx i lives at
    [i % 16, i // 16], replicated across the 8 groups of 16 partitions."""
    n = stream.shape[0]
    arr = stream.reshape(n // 16, 16).T.astype(np.int16)   # [16, n//16]
    return np.tile(arr, (8, 1))                            # [128, n//16]


def prep_inputs(emb_table, head_w, head_b, first, mid, last, inv_i,
                n_tiles=T_CORE // TILE_T):
    """Host-side shard + layout prep.  Returns in_maps for 8 cores."""
    emb = np.asarray(emb_table, dtype=np.float32).copy()
    emb[0] = 0.0  # padding_idx (reference masks id 0; row 0 is zero anyway)
    W = np.asarray(head_w, dtype=np.float32)                 # [768, 768]
    b = np.asarray(head_b, dtype=np.float32)                 # [768]

    # Weight refactoring: fold the first/last head blocks (and the bias)
    # into output-space lookup tables.  T0[0] = b keeps padding_idx
    # semantics exact (E[0] = 0, bias still applies).
    t02 = np.zeros((2 * VOCAB_PAD, O), dtype=BF16)
    t02[:VOCAB] = (emb @ W[0:H] + b).astype(BF16)
    t02[VOCAB:VOCAB_PAD] = b.astype(BF16)                    # never indexed
    t02[VOCAB_PAD:VOCAB_PAD + VOCAB] = (emb @ W[2 * H:3 * H]).astype(BF16)

    # bf16 E table; vocab-major layout: partition = id % 128, chunk = id//128
    tbl16 = np.zeros((VOCAB_PAD, H), dtype=BF16)
    tbl16[:VOCAB] = emb.astype(BF16)
    ecnt_in = np.ascontiguousarray(
        tbl16.reshape(VCH, 128, H).transpose(1, 0, 2)).reshape(128, VCH * H)

    Wmid = W[H:2 * H].astype(BF16)                           # [256, 768]
    wts_in = np.ascontiguousarray(
        Wmid.reshape(KMID, 128, O).transpose(1, 0, 2)).reshape(128, KMID * O)

    inv_i = np.asarray(inv_i)
    uniq = np.unique(inv_i)                          # sorted used uniques
    n_used = uniq.shape[0]
    cap = N_CORES * n_tiles * TILE_T
    assert n_used <= cap, (n_used, cap)
    upad = np.zeros(cap, dtype=inv_i.dtype)
    upad[:n_used] = uniq                             # pad slots -> token 0
    fi = np.asarray(first)[upad].astype(np.int16)    # [cap]
    mi = np.asarray(mid)[upad].astype(np.int16)      # [cap, 12]
    la = (np.asarray(last)[upad] + VOCAB_PAD).astype(np.int16)

    tiles = _tiles(n_tiles)
    in_maps = []
    for c in range(N_CORES):
        base = c * n_tiles * TILE_T
        cols = []
        cnts = []
        tok0 = 0
        for tt in tiles:
            s = slice(base + tok0, base + tok0 + tt)
            tok0 += tt
            cols.append(_wrap_idx(np.concatenate([fi[s], la[s]])))
            cm = np.zeros((VOCAB_PAD, tt), dtype=np.uint8)
            np.add.at(cm, (mi[s].ravel(), np.repeat(np.arange(tt), M)), 1)
            # [4096, tt] -> [128 part, 32 chunk, tt]
            cm = np.ascontiguousarray(
                cm.reshape(VCH, 128, tt).transpose(1, 0, 2))
            cnts.append(_F8_LUT[cm].view(F8))
        idx_in = np.concatenate(cols, axis=1)
        cnt_in = np.concatenate(
            [x.reshape(128, VCH * tt) for x, tt in zip(cnts, tiles)], axis=1)
        in_maps.append({
            "t02": t02, "ecnt": ecnt_in, "wts": wts_in,
            "idx": idx_in, "cnt": np.ascontiguousarray(cnt_in.view(F8)),
        })
    return in_maps


def assemble(per_core, inv_i):
    """Expand per-used-unique device rows to the padded [B, L+2, O] output."""
    inv_i = np.asarray(inv_i)
    uniq = np.unique(inv_i)
    rows = np.concatenate(per_core, axis=0)            # [cap, 768] bf16
    slot = np.searchsorted(uniq, inv_i)                # position -> row slot
    tok = rows[slot].astype(np.float32).reshape(B, L, O)
    full = np.zeros((B, L + 2, O), dtype=np.float32)
    full[:, 1:L + 1, :] = tok
    return full


def kernel(emb_table, head_w, head_b, first, mid, last, inv_i,
           batch, seq_len, _nc=None, _return_raw=False):
    batch = int(batch)
    seq_len = int(seq_len)
    assert batch == B and seq_len == L, (batch, seq_len)
    n_used = np.unique(np.asarray(inv_i)).shape[0]
    n_tiles = -(-n_used // (N_CORES * TILE_T))         # robustness fallback
    n_tiles = max(n_tiles, T_CORE // TILE_T)
    nc = _nc if _nc is not None else _get_nc(n_tiles)
    in_maps = prep_inputs(emb_table, head_w, head_b, first, mid, last, inv_i,
                          n_tiles=n_tiles)
    res = run_bass_kernel_spmd(nc, in_maps, core_ids=list(range(N_CORES)))
    per_core = [r["out"] for r in res.results]         # each [t_core, 768] bf16
    if _return_raw:
        return per_core
    return assemble(per_core, inv_i)
